# revision 2
# baseline (speedup 1.0000x reference)
"""GATv2 graph layer Bass kernel for TRN2, v2 (SPMD across 8 NeuronCores).

Design (per core, edges sorted by dst, dst-sharded):
  xsrc_tab [n_pad, 128] f16 in DRAM: node_emb @ W_src, built in 2048-node
    superblocks with contiguous per-partition writes (host-permuted nodeT).
  xdwT2 [128, npc_pad, 2] f16 in SBUF: (node_own @ W_dst)^T with each value
    duplicated in pairs (so gpsimd ap_gather can fetch d=2 elements).
  Per 128-edge chunk:
    xs   = dma_gather(xsrc_tab, src)          [e,128] f16   (DMA)
    xdT  = ap_gather(xdwT2, dst_local)        [128,e,2] f16 (Pool)
    comb_ps [hid, e+4] f32 PSUM:
      mm(lhsT=emb8, rhs=oh8T_chunk)  (edge-type embedding, one-hot)
      mm(lhsT=xs, rhs=ident)         (transpose-by-matmul: += xs^T)
      mm(lhsT=ident, rhs=xdT[:,:,0]) (+= x_dst^T)
    combT = Prelu(comb_ps[:, :e])  f16 SBUF  (leaky relu, alpha=0.2)
    logits_ps = mm(lhsT=combT, rhs=att_blk) -> [e, 4] (same PSUM bank)
    ex = Exp(logits_ps) -> rhs_t[:, slot, 0:4] f16
    wgt: rhs_t[:, slot, 4:132] = xs * ex (broadcast per head)   (DVE)
    scatter: mm(out=win_ps[wl], lhsT=oh_chunk, rhs=rhs_t[:, slot, :])
  One-hots (oh [e,n] per chunk, oh8T from host) are batched DVE is_equal ops.
  Window flush: agg = msg/sum(ex), @ (W_out*gamma) + residual + LayerNorm,
  with rstd = exp(-0.5*ln(var+eps)) so every activation (Copy/Exp/Ln/Square/
  Prelu) lives in one act table -> no mid-kernel act table reloads.
"""
import numpy as np
from contextlib import ExitStack
from dataclasses import dataclass

import concourse.bass as bass
import concourse.tile as tile
from concourse import bacc, mybir

P = 128
HID = 128
H = 4
HD = 32
NET = 8
EPS_LN = 1e-5
MAXCALL = 4096     # max idxs per dma_gather call
APG_K = 9          # chunks per ap_gather call (multiple of PSK)
PSK = 3            # chunks per PSUM comb bank (3*132*4B < 2KB)
SB = 2048          # xsrc build superblock (nodes)
SPLIT = 24576      # lo/hi src split (multiple of SB)
DEAD = -5.0


@dataclass
class Geo:
    N: int
    n_cores: int
    slab_w: int = 3

    @property
    def npc(self):
        return self.N // self.n_cores

    @property
    def nw(self):
        return (self.npc + P - 1) // P

    @property
    def nslab(self):
        return (self.nw + self.slab_w - 1) // self.slab_w

    @property
    def n_pad(self):
        return ((self.N + SB - 1) // SB) * SB

    @property
    def npc_pad(self):
        return ((self.npc + 511) // 512) * 512


def wrap_idx(idx, cols):
    n = idx.shape[0]
    assert n % 16 == 0
    w = np.zeros((P, cols), dtype=np.int16)
    if n:
        t16 = idx.reshape(n // 16, 16).T
        for g in range(8):
            w[g * 16:(g + 1) * 16, :n // 16] = t16
    return w


def host_prep(g: Geo, node_embeddings, edge_index, edge_type, task_embedding,
              W_src, b_src, W_dst, b_dst, edge_emb, att,
              W_out, b_out, norm_w, norm_b, W_film, b_film):
    src = np.asarray(edge_index[0], dtype=np.int64)
    dst = np.asarray(edge_index[1], dtype=np.int64)
    et = np.asarray(edge_type, dtype=np.int64)
    npc, nw = g.npc, g.nw

    order = np.argsort(dst, kind="stable")
    src, dst, et = src[order], dst[order], et[order]
    core_of = dst // npc

    buckets = {}
    for c in range(g.n_cores):
        m = core_of == c
        cs, cd, ce = src[m], dst[m] - c * npc, et[m]
        for w in range(nw):
            wm = (cd // P) == w
            ws_, wd, we = cs[wm], cd[wm], ce[wm]
            lo = ws_ < SPLIT
            buckets[(c, w, 0)] = (ws_[lo], wd[lo], we[lo])
            buckets[(c, w, 1)] = (ws_[~lo] - SPLIT, wd[~lo], we[~lo])

    caps = np.zeros((nw, 2), dtype=np.int64)
    for w in range(nw):
        for h in range(2):
            mx = max(len(buckets[(c, w, h)][0]) for c in range(g.n_cores))
            caps[w, h] = (mx + P - 1) // P
        if caps[w].sum() == 0:
            caps[w, 0] = 1

    # ---- schedule ---------------------------------------------------------
    sched_slabs = []
    total_chunks = 0
    for s in range(g.nslab):
        ws = list(range(s * g.slab_w, min((s + 1) * g.slab_w, nw)))
        chunks = []              # (wl, half, slot)
        calls = {0: [], 1: []}   # xs gather calls: (slot_off, n_idxs)
        slot = 0
        for h in (0, 1):
            run = 0
            run_start = slot
            for w in ws:
                for _ in range(caps[w, h]):
                    chunks.append((w - ws[0], h, slot))
                    slot += 1
                    run += P
                    if run == MAXCALL:
                        calls[h].append((run_start, run))
                        run, run_start = 0, slot
            if run:
                calls[h].append((run_start, run))
        sched_slabs.append(dict(windows=ws, chunks=chunks, calls=calls,
                                chunk0=total_chunks))
        total_chunks += len(chunks)

    lo_cols = max(16, sum(n for sl in sched_slabs
                          for (_, n) in sl["calls"][0]) // 16)
    hi_cols = max(16, sum(n for sl in sched_slabs
                          for (_, n) in sl["calls"][1]) // 16)
    xd_cols = max(16, total_chunks * P // 16)

    # ---- shared constants -------------------------------------------------
    nodeT = np.asarray(node_embeddings, np.float32).T.astype(np.float16)
    nodeT_perm = np.zeros((HID, g.n_pad), dtype=np.float16)
    j = np.arange(g.n_pad)
    node_of_col = (j // SB) * SB + (j % P) * 16 + (j // P) % 16
    valid = node_of_col < g.N
    nodeT_perm[:, valid] = nodeT[:, node_of_col[valid]]

    emb_eff = (np.asarray(edge_emb, np.float64)
               + np.asarray(b_src, np.float64)[None, :]
               + np.asarray(b_dst, np.float64)[None, :]).astype(np.float16)
    att_blk = np.zeros((HID, H), dtype=np.float16)
    for h in range(H):
        att_blk[h * HD:(h + 1) * HD, h] = np.asarray(att, np.float32)[h]

    ident = np.eye(P, dtype=np.float16)
    iota_row = np.tile(np.arange(P, dtype=np.float16), (P, 1))

    consts = dict(
        nodeT_perm=nodeT_perm,
        W_src=np.asarray(W_src, np.float32).astype(np.float16),
        W_dst=np.asarray(W_dst, np.float32).astype(np.float16),
        W_out=np.asarray(W_out, np.float32).astype(np.float16),
        W_film=np.asarray(W_film, np.float32).astype(np.float16),
        b_film=np.asarray(b_film, np.float32).reshape(1, 2 * HID),
        b_out=np.asarray(b_out, np.float32).reshape(1, HID),
        task=np.asarray(task_embedding, np.float32).reshape(HID, 1)
            .astype(np.float16),
        emb8=emb_eff,                      # [8, HID] f16
        att_blk=att_blk,
        ident=ident,
        iota_row=iota_row,
    )
    skip_norm = bool(np.all(np.asarray(norm_w) == 1.0)
                     and np.all(np.asarray(norm_b) == 0.0))
    if not skip_norm:
        consts["normw"] = np.asarray(norm_w, np.float32).reshape(1, HID)
        consts["normb"] = np.asarray(norm_b, np.float32).reshape(1, HID)

    # ---- per-core arrays --------------------------------------------------
    node_embT = np.asarray(node_embeddings, np.float32).T.astype(np.float16)
    in_maps = []
    for c in range(g.n_cores):
        lo_l, hi_l, xd_l = [], [], []
        dstr = np.full((P, total_chunks), DEAD, dtype=np.float16)
        oh8 = np.zeros((NET, total_chunks * P), dtype=np.float16)
        ci = 0
        for sl in sched_slabs:
            ws0 = sl["windows"][0]
            per_half = {0: [], 1: []}
            nth = {}
            for (wl, h, slot) in sl["chunks"]:
                w = ws0 + wl
                es, ed, ee = buckets[(c, w, h)]
                k = nth.get((wl, h), 0)
                nth[(wl, h)] = k + 1
                sl_src = np.zeros(P, dtype=np.int64)
                sl_dst = np.zeros(P, dtype=np.int64)   # slab-local node id
                n = min(P, max(0, len(es) - k * P))
                if n > 0:
                    sl_src[:n] = es[k * P:k * P + n]
                    sl_dst[:n] = ed[k * P:k * P + n] - ws0 * P
                    dstr[:n, ci] = (ed[k * P:k * P + n] - w * P).astype(
                        np.float16)
                    tt = ee[k * P:k * P + n]
                    oh8[tt, ci * P + np.arange(n)] = 1.0
                per_half[h].append(sl_src)
                xd_l.append(sl_dst)
                ci += 1
            lo_l.extend(per_half[0])
            hi_l.extend(per_half[1])
        lo_i = (np.concatenate(lo_l) if lo_l else np.zeros(0, np.int64))
        hi_i = (np.concatenate(hi_l) if hi_l else np.zeros(0, np.int64))
        xd_i = np.concatenate(xd_l) if xd_l else np.zeros(0, np.int64)
        assert lo_i.max(initial=0) < SPLIT
        assert hi_i.max(initial=0) < 32768
        assert xd_i.max(initial=0) < g.slab_w * P

        m = dict(consts)
        m["node_own"] = np.ascontiguousarray(
            np.asarray(node_embeddings, np.float32)[c * npc:(c + 1) * npc])
        noT = np.zeros((HID, g.npc_pad), dtype=np.float16)
        noT[:, :npc] = node_embT[:, c * npc:(c + 1) * npc]
        m["noT_own"] = noT
        m["lo_idx"] = wrap_idx(lo_i.astype(np.int16), lo_cols)
        m["hi_idx"] = wrap_idx(hi_i.astype(np.int16), hi_cols)
        m["xd_idx"] = wrap_idx(xd_i.astype(np.int16), xd_cols)
        m["dstr"] = dstr
        m["oh8T"] = oh8
        in_maps.append(m)

    sched = dict(slabs=sched_slabs, caps=caps, total_chunks=total_chunks,
                 lo_cols=lo_cols, hi_cols=hi_cols, skip_norm=skip_norm)
    return sched, in_maps


def build_program(g: Geo, sched):
    nc = bacc.Bacc("TRN2", target_bir_lowering=False, debug=False,
                   num_devices=g.n_cores, num_swdge_queues=4)
    f16, f32, bf16 = mybir.dt.float16, mybir.dt.float32, mybir.dt.bfloat16
    AF = mybir.ActivationFunctionType
    OP = mybir.AluOpType
    npc, nw = g.npc, g.nw
    total_chunks = sched["total_chunks"]
    lo_cols, hi_cols = sched["lo_cols"], sched["hi_cols"]
    xd_cols = max(16, total_chunks * P // 16)
    n_sb = g.n_pad // SB
    lo_sb_blocks = SPLIT // SB

    def din(name, shape, dt):
        return nc.dram_tensor(name, shape, dt, kind="ExternalInput").ap()

    nodeT_perm = din("nodeT_perm", [HID, g.n_pad], f16)
    noT_own = din("noT_own", [HID, g.npc_pad], f16)
    node_own = din("node_own", [npc, HID], f32)
    W_src = din("W_src", [HID, HID], f16)
    W_dst = din("W_dst", [HID, HID], f16)
    W_out = din("W_out", [HID, HID], f16)
    W_film = din("W_film", [HID, 2 * HID], f16)
    b_film = din("b_film", [1, 2 * HID], f32)
    b_out = din("b_out", [1, HID], f32)
    task = din("task", [HID, 1], f16)
    emb8 = din("emb8", [NET, HID], f16)
    att_blk = din("att_blk", [HID, H], f16)
    ident_d = din("ident", [P, P], f16)
    iota_d = din("iota_row", [P, P], f16)
    lo_idx = din("lo_idx", [P, lo_cols], mybir.dt.int16)
    hi_idx = din("hi_idx", [P, hi_cols], mybir.dt.int16)
    xd_idx = din("xd_idx", [P, xd_cols], mybir.dt.int16)
    dstr = din("dstr", [P, total_chunks], f16)
    oh8T_d = din("oh8T", [NET, total_chunks * P], f16)
    out = nc.dram_tensor("out", [npc, HID], f32, kind="ExternalOutput").ap()

    xsrc_tab = nc.dram_tensor("xsrc_tab", [g.n_pad, HID], f16).ap()

    with tile.TileContext(nc, trace_sim=False) as tc, ExitStack() as ctx:
        cpool = ctx.enter_context(tc.tile_pool(name="consts", bufs=1))
        bpool = ctx.enter_context(tc.tile_pool(name="build", bufs=2))
        psW = ctx.enter_context(tc.tile_pool(name="psW", bufs=g.slab_w,
                                             space="PSUM"))
        psC = ctx.enter_context(tc.tile_pool(name="psC", bufs=3, space="PSUM"))
        psB = ctx.enter_context(tc.tile_pool(name="psB", bufs=2, space="PSUM"))
        spool = ctx.enter_context(tc.tile_pool(name="slab", bufs=2))
        opool = ctx.enter_context(tc.tile_pool(name="oh", bufs=2))
        gpool = ctx.enter_context(tc.tile_pool(name="xdsel", bufs=3))
        wpool = ctx.enter_context(tc.tile_pool(name="work", bufs=6))
        fpool = ctx.enter_context(tc.tile_pool(name="flush", bufs=2))

        # ---- constants ----------------------------------------------------
        ident = cpool.tile([P, P], f16)
        nc.sync.dma_start(ident[:], ident_d[:])
        iota = cpool.tile([P, P], f16)
        nc.sync.dma_start(iota[:], iota_d[:])
        ones_row = cpool.tile([1, P], f16)
        nc.vector.memset(ones_row[:], 1.0)
        eps_col = cpool.tile([P, 1], f32)
        nc.vector.memset(eps_col[:], EPS_LN)
        magic = cpool.tile([P, 1], mybir.dt.int32)
        nc.vector.memset(magic[:], 0x5F3759DF)

        Ws = cpool.tile([HID, HID], f16)
        nc.sync.dma_start(Ws[:], W_src[:])
        Wd = cpool.tile([HID, HID], f16)
        nc.sync.dma_start(Wd[:], W_dst[:])
        Wo = cpool.tile([HID, HID], f16)
        nc.sync.dma_start(Wo[:], W_out[:])
        Wf = cpool.tile([HID, 2 * HID], f16)
        nc.sync.dma_start(Wf[:], W_film[:])
        emb_sb = cpool.tile([NET, HID], f16)
        nc.sync.dma_start(emb_sb[:], emb8[:])
        att_sb = cpool.tile([HID, H], f16)
        nc.sync.dma_start(att_sb[:], att_blk[:])
        task_sb = cpool.tile([HID, 1], f16)
        nc.sync.dma_start(task_sb[:], task[:])
        bfilm_sb = cpool.tile([1, 2 * HID], f32)
        nc.sync.dma_start(bfilm_sb[:], b_film[:])
        bout_sb = cpool.tile([1, HID], f32)
        nc.sync.dma_start(bout_sb[:], b_out[:])

        # node_own as [P, nw, HID]: partition p, window w -> node w*P+p
        node_own_sb = cpool.tile([P, nw, HID], f32, tag="nodeown")
        tail = npc - (npc // P) * P
        full_w = npc // P
        if tail:
            nc.vector.memset(node_own_sb[:, full_w, :], 0.0)
        if full_w:
            nc.sync.dma_start(
                node_own_sb[:, :full_w, :],
                node_own[:full_w * P, :].rearrange("(w p) h -> p w h", p=P))
        if tail:
            nc.sync.dma_start(node_own_sb[:tail, full_w, :],
                              node_own[full_w * P:, :])

        # ---- FiLM (tanh via exp to stay in one act table) ------------------
        ps_f = psB.tile([1, 2 * HID], f32, space="PSUM", tag="pt")
        nc.tensor.matmul(out=ps_f[:], lhsT=task_sb[:], rhs=Wf[:],
                         start=True, stop=True)
        film = cpool.tile([1, 2 * HID], f32)
        nc.vector.tensor_add(film[:], ps_f[:], bfilm_sb[:])
        e2 = cpool.tile([1, HID], f32)
        nc.scalar.activation(e2[:], film[:, :HID], AF.Exp, scale=2.0)
        e2p = cpool.tile([1, HID], f32)
        nc.vector.tensor_scalar(e2p[:], e2[:], 1.0, None, OP.add)
        rr = cpool.tile([1, HID], f32)
        nc.vector.reciprocal(rr[:], e2p[:])
        # gamma_eff = 1 + 0.5*tanh = 1.5 - 1/(e^{2x}+1)
        gam16 = cpool.tile([1, HID], f16)
        nc.vector.tensor_scalar(gam16[:], rr[:], -1.0, 1.5, OP.mult, OP.add)
        tmpb = cpool.tile([1, HID], f32)
        nc.vector.tensor_mul(tmpb[:], bout_sb[:], gam16[:])
        beta16 = cpool.tile([1, HID], f32)
        nc.vector.tensor_add(beta16[:], tmpb[:], film[:, HID:])
        ps_g = psB.tile([P, HID], f32, space="PSUM", tag="pt")
        nc.tensor.matmul(out=ps_g[:], lhsT=ones_row[:], rhs=gam16[:],
                         start=True, stop=True)
        gam_rep = cpool.tile([P, HID], f16)
        nc.vector.tensor_copy(gam_rep[:], ps_g[:])
        Wosc = cpool.tile([HID, HID], f16)
        nc.vector.tensor_mul(Wosc[:], Wo[:], gam_rep[:])
        beta16h = cpool.tile([1, HID], f16)
        nc.vector.tensor_copy(beta16h[:], beta16[:])
        ps_bt = psB.tile([P, HID], f32, space="PSUM", tag="pt")
        nc.tensor.matmul(out=ps_bt[:], lhsT=ones_row[:], rhs=beta16h[:],
                         start=True, stop=True)
        beta_rep = cpool.tile([P, HID], f32)
        nc.vector.tensor_copy(beta_rep[:], ps_bt[:])
        # fold beta into residual
        nc.vector.tensor_add(
            node_own_sb[:], node_own_sb[:],
            beta_rep[:].unsqueeze(1).broadcast_to([P, nw, HID]))

        if not sched["skip_norm"]:
            nw_dr = din("normw", [1, HID], f32)
            nb_dr = din("normb", [1, HID], f32)
            nw_sb = cpool.tile([1, HID], f32)
            nc.sync.dma_start(nw_sb[:], nw_dr[:])
            nb_sb = cpool.tile([1, HID], f32)
            nc.sync.dma_start(nb_sb[:], nb_dr[:])
            ones32 = cpool.tile([1, P], f32)
            nc.vector.memset(ones32[:], 1.0)
            ps_w = psB.tile([P, HID], f32, space="PSUM", tag="pt")
            nc.tensor.matmul(out=ps_w[:], lhsT=ones32[:], rhs=nw_sb[:],
                             start=True, stop=True)
            w_rep = cpool.tile([P, HID], f32)
            nc.vector.tensor_copy(w_rep[:], ps_w[:])
            ps_b = psB.tile([P, HID], f32, space="PSUM", tag="pt")
            nc.tensor.matmul(out=ps_b[:], lhsT=ones32[:], rhs=nb_sb[:],
                             start=True, stop=True)
            b_rep = cpool.tile([P, HID], f32)
            nc.vector.tensor_copy(b_rep[:], ps_b[:])

        # ---- xdwT2 table in SBUF: [128, npc_pad, 2] f16 (dup pairs) --------
        xdwT2 = cpool.tile([P, g.npc_pad, 2], f16, tag="xdw")
        for blk in range(g.npc_pad // 512):
            nt = bpool.tile([HID, 512], f16, tag="not")
            nc.sync.dma_start(nt[:], noT_own[:, blk * 512:(blk + 1) * 512])
            ps = psB.tile([P, 512], f32, space="PSUM", tag="pt")
            nc.tensor.matmul(out=ps[:], lhsT=Wd[:], rhs=nt[:],
                             start=True, stop=True, skip_group_check=True)
            pv = ps[:].unsqueeze(2).broadcast_to([P, 512, 2])
            dstv = xdwT2[:, blk * 512:(blk + 1) * 512, :]
            if blk % 2 == 0:
                nc.scalar.activation(dstv, pv, AF.Copy)
            else:
                nc.vector.tensor_copy(dstv, pv)

        # ---- xsrc table (DRAM), superblocks of 2048 nodes ------------------
        for blk in range(n_sb):
            nt = bpool.tile([HID, SB], f16, tag="nt")
            nc.sync.dma_start(nt[:], nodeT_perm[:, blk * SB:(blk + 1) * SB])
            stg = bpool.tile([P, 16, HID], f16, tag="stg")
            for b in range(4):
                ps = psB.tile([P, 512], f32, space="PSUM", tag="pt")
                for s in range(4):
                    m = 4 * b + s
                    nc.tensor.matmul(out=ps[:, s * HID:(s + 1) * HID],
                                     lhsT=nt[:, m * P:(m + 1) * P], rhs=Ws[:],
                                     start=True, stop=True,
                                     skip_group_check=True)
                pv = ps[:].rearrange("p (s h) -> p s h", s=4)
                if b % 2 == 0:
                    nc.scalar.activation(stg[:, 4 * b:4 * b + 4, :], pv,
                                         AF.Copy)
                else:
                    nc.vector.tensor_copy(stg[:, 4 * b:4 * b + 4, :], pv)
            nc.sync.dma_start(
                xsrc_tab[blk * SB:(blk + 1) * SB, :]
                .rearrange("(p m) h -> p m h", p=P), stg[:])

        # ---- idx staging ---------------------------------------------------
        lo_sb = cpool.tile([P, lo_cols], mybir.dt.int16, tag="loidx")
        nc.sync.dma_start(lo_sb[:], lo_idx[:])
        hi_sb = cpool.tile([P, hi_cols], mybir.dt.int16, tag="hiidx")
        nc.sync.dma_start(hi_sb[:], hi_idx[:])
        xdi_sb = cpool.tile([P, xd_cols], mybir.dt.int16, tag="xdidx")
        nc.sync.dma_start(xdi_sb[:], xd_idx[:])
        dstr_sb = cpool.tile([P, total_chunks], f16, tag="dstr")
        nc.sync.dma_start(dstr_sb[:], dstr[:])

        off16 = {0: 0, 1: 0}
        _qctr = [0]

        def nextq():
            _qctr[0] = (_qctr[0] + 1) % 4
            return _qctr[0]

        # ---- edge slabs ----------------------------------------------------
        for s, sl in enumerate(sched["slabs"]):
            ws = sl["windows"]
            nwin = len(ws)
            chunks = sl["chunks"]
            C = len(chunks)
            c0 = sl["chunk0"]

            # xs gathers (DMA, SWDGE via Pool)
            xs_t = spool.tile([P, C, HID], f16, tag="xs")
            for h in (0, 1):
                base = SPLIT if h == 1 else 0
                top = SPLIT if h == 0 else g.n_pad
                idx_sb = lo_sb if h == 0 else hi_sb
                for (slot_off, n) in sl["calls"][h]:
                    if n == 0:
                        continue
                    nc.gpsimd.dma_gather(
                        out_ap=xs_t[:, slot_off:slot_off + n // P, :],
                        in_ap=xsrc_tab[base:top, :],
                        idxs_ap=idx_sb[:, off16[h]:off16[h] + n // 16],
                        num_idxs=n, num_idxs_reg=n, elem_size=HID,
                        single_packet=(n <= 1024), queue_num=nextq(),
                    )
                    off16[h] += n // 16

            # xd selection via ap_gather (Pool), groups of APG_K chunks.
            # Slab-local table view + u32-packed pairs keep the per-call
            # ap cost at max(out, in_view) elements.
            w0 = ws[0]
            span = nwin * P
            tab_view = xdwT2[:, w0 * P:w0 * P + span, :].bitcast(
                mybir.dt.uint32)
            xd_sels = []
            for g0 in range(0, C, APG_K):
                gn = min(APG_K, C - g0)
                xd_sel = gpool.tile([P, APG_K * P], mybir.dt.uint32,
                                    tag="xdsel")
                nc.gpsimd.ap_gather(
                    out_ap=xd_sel[:, :gn * P],
                    in_ap=tab_view,
                    idxs_ap=xdi_sb[:, (c0 + g0) * P // 16:
                                   (c0 + g0 + gn) * P // 16],
                    channels=P, num_elems=span, d=1, num_idxs=gn * P)
                xd_sels.append((g0, gn, xd_sel))

            # one-hot scatter matrix for the slab (DVE, one op)
            oh_t = opool.tile([P, C, P], f16, tag="oh")
            nc.vector.tensor_tensor(
                oh_t[:],
                iota[:].unsqueeze(1).broadcast_to([P, C, P]),
                dstr_sb[:, c0:c0 + C].unsqueeze(2).broadcast_to([P, C, P]),
                OP.is_equal)

            # edge-type one-hot rows (host-built)
            oh8_sl = opool.tile([NET, C * P], f16, tag="oh8")
            nc.sync.dma_start(oh8_sl[:], oh8T_d[:, c0 * P:(c0 + C) * P])

            win_ps = [psW.tile([P, 4 + HID], f32, space="PSUM", tag="win",
                               name=f"win{s}_{i}")
                      for i in range(nwin)]
            n_per_win = [0] * nwin
            for (wl, h, slot) in chunks:
                n_per_win[wl] += 1
            seen = [0] * nwin

            rhs_t = spool.tile([P, C, 4 + HID], f16, tag="rhs")

            # per-chunk compute, PSK chunks share one PSUM bank so the
            # Prelu/Exp evacuations amortize the PSUM-access penalty
            for (g0, gn, xd_sel) in xd_sels:
                xdv = xd_sel[:].bitcast(f16).rearrange(
                    "p (c e two) -> p c e two", c=APG_K, two=2)
                for q0 in range(0, gn, PSK):
                    qn = min(PSK, gn - q0)
                    cps = psC.tile([P, PSK, P + 4], f32, space="PSUM",
                                   tag="cps")
                    for k in range(qn):
                        slot = g0 + q0 + k
                        nc.tensor.matmul(
                            out=cps[:, k, :P],
                            lhsT=emb_sb[:],
                            rhs=oh8_sl[:, slot * P:(slot + 1) * P],
                            start=True, stop=False, skip_group_check=True)
                        nc.tensor.matmul(
                            out=cps[:, k, :P], lhsT=xs_t[:, slot, :],
                            rhs=ident[:],
                            start=False, stop=False, skip_group_check=True)
                        nc.tensor.matmul(
                            out=cps[:, k, :P], lhsT=ident[:],
                            rhs=xdv[:, q0 + k, :, 0],
                            start=False, stop=True, skip_group_check=True)
                    combT = wpool.tile([HID, PSK, P], f16, tag="combT")
                    nc.scalar.activation(combT[:, :qn, :], cps[:, :qn, :P],
                                         AF.Prelu, alpha=0.2)
                    for k in range(qn):
                        nc.tensor.matmul(
                            out=cps[:, k, P:P + 4], lhsT=combT[:, k, :],
                            rhs=att_sb[:],
                            start=True, stop=True, skip_group_check=True)
                    nc.scalar.activation(rhs_t[:, g0 + q0:g0 + q0 + qn, 0:4],
                                         cps[:, :qn, P:P + 4], AF.Exp)
                # weighted messages for the group (DVE)
                nc.vector.tensor_mul(
                    rhs_t[:, g0:g0 + gn, 4:].rearrange(
                        "p c (h d) -> p c h d", h=H),
                    xs_t[:, g0:g0 + gn, :].rearrange(
                        "p c (h d) -> p c h d", h=H),
                    rhs_t[:, g0:g0 + gn, 0:4].unsqueeze(3)
                    .broadcast_to([P, gn, H, HD]))
                # scatter the group's chunks
                for k in range(gn):
                    slot = g0 + k
                    wl = chunks[slot][0]
                    first = seen[wl] == 0
                    last = seen[wl] == n_per_win[wl] - 1
                    seen[wl] += 1
                    nc.tensor.matmul(out=win_ps[wl][:],
                                     lhsT=oh_t[:, slot, :],
                                     rhs=rhs_t[:, slot, :],
                                     start=first, stop=last,
                                     skip_group_check=True)

            # ---- flush windows --------------------------------------------
            vs_slab = fpool.tile([P, nwin], f32, tag="vs", name=f"vss{s}")
            cen_l = []
            for wl, w in enumerate(ws):
                pw = win_ps[wl]
                sums = fpool.tile([P, 4], f32, tag="sums")
                nc.vector.tensor_scalar(sums[:], pw[:, 0:4], 1e-12, None,
                                        OP.max)
                rec = fpool.tile([P, 4], f32, tag="rec")
                nc.vector.reciprocal(rec[:], sums[:])
                aggn = fpool.tile([P, HID], f16, tag="aggn")
                nc.vector.tensor_mul(
                    aggn[:].rearrange("p (h d) -> p h d", h=H),
                    pw[:, 4:].rearrange("p (h d) -> p h d", h=H),
                    rec[:].unsqueeze(2).broadcast_to([P, H, HD]))
                psT = psB.tile([P, P], f16, space="PSUM", tag="pt")
                nc.tensor.transpose(out=psT[:], in_=aggn[:], identity=ident[:])
                aggT = fpool.tile([HID, P], f16, tag="aggT")
                nc.scalar.activation(aggT[:], psT[:], AF.Copy)
                po = psB.tile([P, HID], f32, space="PSUM", tag="pt")
                nc.tensor.matmul(out=po[:], lhsT=aggT[:], rhs=Wosc[:],
                                 start=True, stop=True)
                y = fpool.tile([P, HID], f32, tag="y", name=f"y{s}_{wl}",
                               bufs=g.slab_w + 1)
                nc.vector.tensor_add(y[:], po[:], node_own_sb[:, w, :])
                mus = fpool.tile([P, 1], f32, tag="mus")
                nc.vector.tensor_reduce(mus[:], y[:], axis=mybir.AxisListType.X,
                                        op=OP.add)
                mu = fpool.tile([P, 1], f32, tag="mu")
                nc.vector.tensor_scalar(mu[:], mus[:], 1.0 / HID, None,
                                        OP.mult)
                cen = fpool.tile([P, HID], f32, tag="cen", name=f"cen{s}_{wl}",
                                 bufs=g.slab_w + 1)
                nc.vector.tensor_scalar(cen[:], y[:], mu[:], None, OP.subtract)
                cen_l.append(cen)
                sq = fpool.tile([P, HID], f16, tag="sq")
                nc.scalar.activation(sq[:], cen[:], AF.Square,
                                     accum_out=vs_slab[:, wl:wl + 1])
            # rstd = 1/sqrt(var+eps) via bit-trick + 2 Newton steps, all on
            # DVE -- keeps the act table pinned to one set for the whole
            # kernel (Copy/Exp/Prelu/Square only).
            v = fpool.tile([P, nwin], f32, tag="v", name=f"v{s}")
            nc.vector.tensor_scalar(v[:], vs_slab[:], 1.0 / HID, EPS_LN,
                                    OP.mult, OP.add)
            ih = fpool.tile([P, nwin], mybir.dt.int32, tag="ih", name=f"ih{s}")
            nc.vector.tensor_scalar(ih[:], v[:].bitcast(mybir.dt.int32),
                                    1, None, OP.logical_shift_right)
            yb = fpool.tile([P, nwin], mybir.dt.int32, tag="yb", name=f"yb{s}")
            nc.vector.tensor_tensor(
                yb[:], magic[:].broadcast_to([P, nwin]), ih[:], OP.subtract)
            rstd = yb[:].bitcast(f32)
            for _ in range(2):
                h2 = fpool.tile([P, nwin], f32, tag="h2")
                nc.vector.tensor_mul(h2[:], rstd, rstd)
                nc.vector.tensor_mul(h2[:], h2[:], v[:])
                nc.vector.tensor_scalar(h2[:], h2[:], -0.5, 1.5,
                                        OP.mult, OP.add)
                nc.vector.tensor_mul(rstd, rstd, h2[:])
            for wl, w in enumerate(ws):
                yn = fpool.tile([P, HID], f32, tag="yn")
                nc.vector.tensor_scalar(yn[:], cen_l[wl][:],
                                        yb[:, wl:wl + 1].bitcast(f32), None,
                                        OP.mult)
                if not sched["skip_norm"]:
                    nc.vector.tensor_mul(yn[:], yn[:], w_rep[:])
                    nc.vector.tensor_add(yn[:], yn[:], b_rep[:])
                rows = min(P, npc - w * P)
                nc.sync.dma_start(out[w * P:w * P + rows, :], yn[:rows, :])

    nc.compile()
    return nc


# ---------------------------------------------------------------------------
_CACHE = {}


def kernel(**inputs):
    N = int(np.asarray(inputs["node_embeddings"]).shape[0])
    n_cores = 8
    g = Geo(N=N, n_cores=n_cores, slab_w=3)

    sched, in_maps = host_prep(g, **{k: np.asarray(v)
                                     for k, v in inputs.items()})

    key = (N, sched["total_chunks"],
           tuple(int(x) for x in sched["caps"].ravel()), sched["skip_norm"])
    if key not in _CACHE:
        _CACHE[key] = build_program(g, sched)
    nc = _CACHE[key]

    from concourse.bass_utils import run_bass_kernel_spmd
    res = run_bass_kernel_spmd(nc, in_maps, core_ids=list(range(n_cores)))
    out = np.concatenate([res.results[c]["out"] for c in range(n_cores)],
                         axis=0)
    return out.astype(np.float32)
